# revision 17
# baseline (speedup 1.0000x reference)
"""LDA head (segment-reduce + Mahalanobis scores) on 8 Trainium2 NeuronCores.

Strategy (single SPMD NEFF on 8 cores, fully replicated stats — no
cross-core communication):
  - Every core reads the full batch and computes the segment stats for ALL
    1000 classes:  psSS = M^T @ onehot  with M = [z | z^2] in fp16, i.e. 16
    accumulating PE matmuls (one per 128-row batch tile) producing
    [S1^T; S2^T] = (128, 1000) fp32 in PSUM — already in the d-partition
    layout the score phase needs, so no transposes and no collective at all.
    (The previous designs exchanged class-sharded stats via a CC AllGather /
    remote SBUF DMAs; both cost 50-150us in collective machinery and launch-
    skew barrier waits.  Recomputing on every core costs ~15us of fp16 PE.)
  - Class counts come from a ones^T @ ohsum matmul (ohsum = sum of the 16
    one-hot tiles on DVE).  fp16 is exact for the one-hot compare (labels
    < 2048) and for the counts; z in fp16 only affects mean/var by ~1e-3
    relative, far inside the 2e-2 gate.
  - Per-column (per-class) scaling rows (1/counts, count correction) are
    broadcast across partitions with ones^T @ row PE matmuls.
  - Scores are batch-sharded: core k computes output rows [256k, 256k+256) as
    out = [z^T; 1]^T @ [prec*mean; log prior - 0.5*r]  (+ -0.5*q_b fused into
    the PSUM->SBUF copy as a per-partition activation bias), with z in fp32.

kernel(z, y) takes the full inputs and returns the full (2048, 1000) output.
"""

import sys
import numpy as np

if "/opt/trn_rl_repo" not in sys.path:
    sys.path.insert(0, "/opt/trn_rl_repo")

import concourse.bacc as bacc
import concourse.bass as bass
import concourse.mybir as mybir
from concourse import tile
from concourse.bass_utils import run_bass_kernel_spmd

B, C, D = 2048, 1000, 64
NCORES = 8
NT = B // 128               # 16 batch tiles
BL = B // NCORES            # 256 output rows per core
JT = BL // 128              # 2 local batch tiles
CH = C // 2                 # 500-column halves (PSUM bank = 2KB)
EPS_STATS = 1e-5
EPS_PREC = 1e-6
TSUM = float(np.float32(B) + np.float32(C * EPS_STATS))   # counts.sum()
FP = mybir.dt.float32
FH = mybir.dt.float16
AF = mybir.ActivationFunctionType
ALU = mybir.AluOpType


def build_program():
    nc = bacc.Bacc("TRN2", target_bir_lowering=False, debug=False,
                   num_devices=NCORES)

    z_in = nc.dram_tensor("z_in", [B, D], FP, kind="ExternalInput")
    ycols = nc.dram_tensor("ycols", [128, NT], FP, kind="ExternalInput")
    cvals = nc.dram_tensor("cvals", [128, C], FH, kind="ExternalInput")
    zloc = nc.dram_tensor("zloc", [128, JT, D], FP, kind="ExternalInput")
    ident = nc.dram_tensor("ident", [128, 128], FP, kind="ExternalInput")
    out = nc.dram_tensor("out_loc", [BL, C], FP, kind="ExternalOutput")

    with tile.TileContext(nc) as tc:
        with tc.tile_pool(name="sb", bufs=1) as pool, \
             tc.tile_pool(name="pss", bufs=1, space="PSUM") as ppS, \
             tc.tile_pool(name="ps", bufs=6, space="PSUM") as pp:

            # ---- input DMAs -------------------------------------------------
            cv = pool.tile([128, C], FH)
            nc.sync.dma_start(cv[:], cvals[:, :])
            yc = pool.tile([128, NT], FP)
            nc.sync.dma_start(yc[:], ycols[:, :])
            idn = pool.tile([128, 128], FP)
            nc.sync.dma_start(idn[:], ident[:, :])
            zl = pool.tile([128, JT, D], FP)
            nc.sync.dma_start(zl[:], zloc[:, :, :])
            Mf = pool.tile([128, NT, D], FP)
            # batch row p*NT+t -> partition p: one contiguous 4KB run/partition
            nc.sync.dma_start(Mf[:, :, :],
                              z_in[:, :].rearrange("(p t) d -> p t d", p=128))

            # ---- phase A: replicated segment stats for all classes ---------
            Mh = pool.tile([128, NT, D + 1], FH)   # [z | 1] in fp16
            nc.vector.tensor_copy(Mh[:, :, 0:D], Mf[:, :, :])
            nc.vector.memset(Mh[:, :, D:D + 1], 1.0)

            # one-hot over all 1000 classes (fp16 exact for labels < 2048);
            # all on DVE: gpsimd shares SBUF ports with DVE
            oh = pool.tile([128, NT, C], FH)
            for t in range(NT):
                nc.vector.tensor_scalar(oh[:, t, :], cv[:], yc[:, t:t + 1],
                                        None, ALU.is_equal)

            # psSS = [S1^T; counts]  (65 rows, 1000 classes);
            # two 500-col halves: a matmul output cannot span PSUM banks
            psh0 = ppS.tile([65, CH], FP, tag="pss0")
            psh1 = ppS.tile([65, CH], FP, tag="pss1")
            psh = [psh0, psh1]
            for t in range(NT):
                for h in range(2):
                    nc.tensor.matmul(psh[h][:], lhsT=Mh[:, t, :],
                                     rhs=oh[:, t, h * CH:(h + 1) * CH],
                                     start=(t == 0), stop=(t == NT - 1))

            # sum_b z^2 over the full batch (class-independent: every batch
            # row lands in exactly one class, so sum_c S2T[d,c] = sum_b z^2)
            sq32 = pool.tile([128, NT, D], FP)
            nc.scalar.activation(sq32[:], Mf[:], AF.Square)
            zs2 = pool.tile([128, D], FP)
            nc.vector.reduce_sum(zs2[:], sq32[:].rearrange("p t d -> p d t"),
                                 axis=mybir.AxisListType.X)
            ones128 = pool.tile([128, 1], FP)
            nc.vector.memset(ones128[:], 1.0)
            psS2 = pp.tile([D, 1], FP, tag="ps")
            nc.tensor.matmul(psS2[:], lhsT=zs2[:], rhs=ones128[:],
                             start=True, stop=True)
            s2s = pool.tile([64, 1], FP)
            nc.vector.tensor_copy(s2s[:], psS2[:])

            # stats to SBUF (engines may read at most one PSUM input)
            S = pool.tile([65, C], FP)
            for h in range(2):
                nc.scalar.copy(S[:, h * CH:(h + 1) * CH], psh[h][:])

            # per-class rows: counts, 1/cnt, log prior, g = (cnt+2e)/cnt
            # (applied to S1T*meanT, which already carries one 1/cnt);
            # reciprocal on the scalar engine: ~6x faster per element than DVE
            cnt = pool.tile([1, C], FP)            # counts + eps
            nc.vector.tensor_scalar_add(cnt[:], S[64:65, :], EPS_STATS)
            lncnt = pool.tile([1, C], FP)
            nc.scalar.activation(lncnt[:], cnt[:], AF.Ln)
            rcp = pool.tile([1, C], FP)            # 1/cnt = exp(-ln cnt)
            nc.scalar.activation(rcp[:], lncnt[:], AF.Exp, scale=-1.0)
            beta = pool.tile([1, C], FP)           # ln(cnt/TSUM)
            nc.vector.tensor_scalar_add(beta[:], lncnt[:],
                                        -float(np.log(TSUM)))
            cnt2 = pool.tile([1, C], FP)           # cnt + 2*eps
            nc.vector.tensor_scalar_add(cnt2[:], cnt[:], EPS_STATS)

            # broadcast the rows across partitions (ones^T @ row); the cnt2
            # broadcast has no Ln/Exp dependency so it overlaps the rcp chain,
            # and meanT^2 doubles as the score phase's msq
            onesr = pool.tile([1, 128], FP)
            nc.vector.memset(onesr[:], 1.0)
            meanT = pool.tile([64, C], FP)          # = msb of the score phase
            msq = pool.tile([64, C], FP)            # meanT^2
            corr = pool.tile([64, C], FP)           # meanT^2*(cnt+2e)
            for h in range(2):
                hs = slice(h * CH, (h + 1) * CH)
                psGB = pp.tile([64, CH], FP, tag="ps")
                nc.tensor.matmul(psGB[:], lhsT=onesr[:, 0:64], rhs=cnt2[:, hs],
                                 start=True, stop=True)
                psRB = pp.tile([64, CH], FP, tag="ps")
                nc.tensor.matmul(psRB[:], lhsT=onesr[:, 0:64], rhs=rcp[:, hs],
                                 start=True, stop=True)
                nc.vector.tensor_tensor(meanT[:, hs], S[0:64, hs],
                                        psRB[:], ALU.mult)
                nc.vector.tensor_tensor(msq[:, hs], meanT[:, hs],
                                        meanT[:, hs], ALU.mult)
                nc.vector.tensor_tensor(corr[:, hs], msq[:, hs],
                                        psGB[:], ALU.mult)

            # pooled covariance: (sum_b z^2 - sum_c S1T*meanT*g)/TSUM + eps
            bsum = pool.tile([64, 1], FP)
            nc.vector.reduce_sum(bsum[:], corr[:, :],
                                 axis=mybir.AxisListType.X)
            pooled = pool.tile([64, 1], FP)
            nc.vector.tensor_tensor(pooled[:], s2s[:], bsum[:], ALU.subtract)
            nc.vector.tensor_scalar(pooled[:], pooled[:], 1.0 / TSUM,
                                    EPS_STATS, ALU.mult, ALU.add)
            pmax = pool.tile([64, 1], FP)
            nc.vector.tensor_scalar_max(pmax[:], pooled[:], EPS_PREC)
            prec = pool.tile([64, 1], FP)
            nc.vector.reciprocal(prec[:], pmax[:])

            # ---- phase B: batch-sharded Mahalanobis scores ------------------
            # local z^T for the score matmuls
            zTq = pool.tile([65, 256], FH)
            nc.vector.memset(zTq[64:65, :], 1.0)
            for j in range(JT):
                psZ = pp.tile([64, 128], FP, tag="ps")
                nc.tensor.transpose(psZ[:], zl[:, j, :], idn[:, :])
                nc.scalar.copy(zTq[0:64, j * 128:(j + 1) * 128], psZ[:])
            zsq = pool.tile([128, JT, D], FP)
            nc.scalar.activation(zsq[:], zl[:], AF.Square)

            # PE warm-up (HAM needs a busy stretch before the score matmuls)
            junkps = pp.tile([64, 64], FP, tag="ps")
            for w in range(10):
                nc.tensor.matmul(junkps[:], lhsT=idn[0:64, 0:64],
                                 rhs=meanT[:, w * 64:w * 64 + 64],
                                 start=True, stop=True)

            # prec broadcast across partitions for the q computation
            psPR = pp.tile([1, 64], FP, tag="ps")
            nc.tensor.transpose(psPR[:], prec[:], idn[0:64, 0:64])
            prow = pool.tile([1, 64], FP)
            nc.vector.tensor_copy(prow[:], psPR[:])
            psPB = pp.tile([128, 64], FP, tag="ps")
            nc.tensor.matmul(psPB[:], lhsT=onesr[:], rhs=prow[:],
                             start=True, stop=True)
            precbc = pool.tile([128, 64], FP)
            nc.vector.tensor_copy(precbc[:], psPB[:])

            V = pool.tile([65, C], FH)              # [prec*mean; beta - r/2]
            nc.vector.tensor_scalar(V[0:64, :], meanT[:], prec[:], None,
                                    ALU.mult)
            rrow = pool.tile([1, C], FP)
            for h in range(2):
                psR = pp.tile([1, CH], FP, tag="ps")
                nc.tensor.matmul(psR[:], lhsT=prec[:],
                                 rhs=msq[:, h * CH:(h + 1) * CH],
                                 start=True, stop=True)
                nc.scalar.activation(rrow[:, h * CH:(h + 1) * CH], psR[:],
                                     AF.Copy, scale=-0.5)
            nc.vector.tensor_tensor(rrow[:], rrow[:], beta[:], ALU.add)
            nc.scalar.copy(V[64:65, :], rrow[:])

            junk = pool.tile([128, 64], FP)
            qraw = pool.tile([128, JT], FP)
            qsb = pool.tile([128, JT], FP)
            for j in range(JT):
                nc.vector.tensor_tensor(junk[:], zsq[:, j, :], precbc[:],
                                        ALU.mult)
                nc.vector.reduce_sum(qraw[:, j:j + 1], junk[:],
                                     axis=mybir.AxisListType.X)
            nc.vector.tensor_scalar_mul(qsb[:], qraw[:], -0.5)

            for j in range(JT):
                outj = pool.tile([128, C], FP, tag=f"outsb{j}")
                for h in range(2):
                    psO = pp.tile([128, CH], FP, tag="ps")
                    nc.tensor.matmul(psO[:],
                                     lhsT=zTq[:, j * 128:(j + 1) * 128],
                                     rhs=V[:, h * CH:(h + 1) * CH],
                                     start=True, stop=True)
                    if h == 0:
                        nc.scalar.activation(outj[:, h * CH:(h + 1) * CH],
                                             psO[:], AF.Identity,
                                             bias=qsb[:, j:j + 1], scale=1.0)
                    else:
                        # split the PSUM->SBUF bias-adds across scalar and DVE
                        nc.vector.tensor_scalar(
                            outj[:, h * CH:(h + 1) * CH], psO[:],
                            qsb[:, j:j + 1], None, ALU.add)
                nc.sync.dma_start(out[j * 128:(j + 1) * 128, :], outj[:])

    nc.compile()
    return nc


_NC_CACHE = None


def _get_program():
    global _NC_CACHE
    if _NC_CACHE is None:
        _NC_CACHE = build_program()
    return _NC_CACHE


def make_in_maps(z, y):
    z = np.ascontiguousarray(np.asarray(z, dtype=np.float32))
    yf = np.asarray(y).astype(np.float32)          # labels < 1000, exact
    ycols_np = np.ascontiguousarray(yf.reshape(128, NT))
    cvals_np = np.ascontiguousarray(
        np.broadcast_to(np.arange(C, dtype=np.float16), (128, C)))
    ident_np = np.eye(128, dtype=np.float32)
    in_maps = []
    for k in range(NCORES):
        zloc_np = np.ascontiguousarray(
            z[k * BL:(k + 1) * BL].reshape(JT, 128, D).transpose(1, 0, 2))
        in_maps.append({
            "z_in": z,
            "ycols": ycols_np,
            "cvals": cvals_np,
            "zloc": zloc_np,
            "ident": ident_np,
        })
    return in_maps


def run(z, y, trace=False, **kwargs):
    nc = _get_program()
    res = run_bass_kernel_spmd(nc, make_in_maps(z, y), list(range(NCORES)),
                               trace=trace, **kwargs)
    full = np.concatenate([res.results[k]["out_loc"] for k in range(NCORES)],
                          axis=0)
    return full, res


def kernel(z, y):
    full, _ = run(z, y, trace=False)
    return full


if __name__ == "__main__":
    rng = np.random.default_rng(0)
    z = rng.standard_normal((B, D), dtype=np.float32)
    y = rng.integers(0, C, size=(B,)).astype(np.int64)
    out = kernel(z, y)
    print("out", out.shape, out.dtype, out[0, :4])
